# revision 16
# baseline (speedup 1.0000x reference)
"""Trainium2 Bass kernel for nn_Cross_Attention (8-core data-parallel over batch).

Per batch item (one NeuronCore):
  kvf  = conv1x1(kv, qkv1_w)                    # [384, H, W]
  kvd  = depthwise3x3(kvf, qkv2_w, pad=1)       # [384, H, W]
  k, v = split(kvd); q/k L2-normalized over hw per channel row
  attn = softmax(scale * qn @ kn^T)             # per-head 24x24 blocks
  out  = proj1x1(attn @ v, proj_w)              # [192, H, W]

Structure (v2, tensor-lean):
  - conv1 runs per-mc (3 slabs of 128 out-channels) from an SBUF-resident
    f16 copy of kv; 3 of the 9 depthwise taps accumulate directly into the
    conv1 PSUM group as diagonal-weight matmuls (contiguous rhs); the other
    6 taps are scalar_tensor_tensor FMAs split by image-row range across
    the vector and gpsimd engines (no cross-engine serialization).
  - attn@v and the 1x1 projection are fused: MT = attn^T @ proj^T is built
    once ([192,192], 4 small matmuls), then out = MT^T @ v per pixel tile.
  - q/k norms are folded into the softmax scaling (Gram runs on raw q,
    k is scaled by 1/|k| before transpose), so Gram accumulates in PSUM
    while q streams in.
  - k and q are moved to pixel-major layout with DMA transposes in 4
    quarter-image chunks, pipelined with the Gram matmuls.
"""

import sys

sys.path.insert(0, "/opt/trn_rl_repo")

import numpy as np

import concourse.bass as bass
import concourse.tile as tile
from concourse import bacc, mybir
from concourse.bass_utils import run_bass_kernel_spmd
from concourse.bass_interp import get_hw_module

F32 = mybir.dt.float32
import ml_dtypes
BF16NP = ml_dtypes.bfloat16
F16 = mybir.dt.bfloat16
AF = mybir.ActivationFunctionType
OP = mybir.AluOpType

C = 192          # input channels
C2 = 384         # conv1 output channels
HEADS = 8
CD = C // HEADS
W = 128
H = 128
HWTOT = H * W    # 16384
PT = 512         # pixels per matmul tile
RPT = PT // W    # 4 rows per tile
NT = HWTOT // PT # 32 tiles
QCH = HWTOT // 4 # quarter-image pixels (4096)
EPS = 1e-12

# depthwise tap split; wi = (dr+1)*3 + (dc+1)
TENSOR_TAPS = [(0, 0), (-1, -1), (1, 1), (0, -1), (0, 1)]
FMA_TAPS = [(-1, 0), (1, 0), (-1, 1), (1, -1)]  # vector in-place FMA taps


def sl(nt, size=PT):
    return slice(nt * size, (nt + 1) * size)


def emit_kernel(tc, io, debug=False):
    nc = tc.nc
    kv, q = io["kv"], io["q"]
    w1t, w2s, w2dg, wpt = io["w1t"], io["w2s"], io["w2dg"], io["wpt"]
    mask, scale192, out = io["mask"], io["scale192"], io["out"]

    from contextlib import ExitStack
    _stack = ExitStack()
    wp = _stack.enter_context(tc.tile_pool(name="weights", bufs=1))
    sml = _stack.enter_context(tc.tile_pool(name="small", bufs=1))
    big = _stack.enter_context(tc.tile_pool(name="big", bufs=1))

    # ---- weights to SBUF ----
    w1ta = wp.tile([128, C2], F16); nc.sync.dma_start(w1ta[:], w1t[0:128, :])
    w1tb = wp.tile([64, C2], F16); nc.sync.dma_start(w1tb[:], w1t[128:C, :])
    wpta = wp.tile([128, C], F16); nc.sync.dma_start(wpta[:], wpt[0:128, :])
    wptb = wp.tile([64, C], F16); nc.sync.dma_start(wptb[:], wpt[128:C, :])
    maska = wp.tile([128, C], F32); nc.sync.dma_start(maska[:], mask[0:128, :])
    maskb = wp.tile([64, C], F32); nc.sync.dma_start(maskb[:], mask[128:C, :])
    sca = wp.tile([128, 1], F32); nc.sync.dma_start(sca[:], scale192[0:128, :])
    scb = wp.tile([64, 1], F32); nc.sync.dma_start(scb[:], scale192[128:C, :])
    w2sb = wp.tile([128, 27], F16); nc.sync.dma_start(w2sb[:], w2s[:, :])
    NTT = len(TENSOR_TAPS)
    w2dgb = wp.tile([128, 3 * NTT, 128], F16)
    nc.sync.dma_start(w2dgb[:], w2dg[:, :, :])

    # ---- persistent big tensors ----
    kvd = [big.tile([128, HWTOT], F16, name=f"kvd{i}") for i in range(3)]

    # ---- phases 0+1: conv1 + depthwise ----
    # mc=0 converts kv f32 tiles to f16 inline and stores the f16 copy to
    # DRAM (kv16d); mc=1,2 stream it back. kvf double-buffers across mc so
    # the next mc's conv (tensor) overlaps this mc's FMA taps (vector).
    # Tiles are processed in groups of GS so each diagonal tap weight is
    # reused across GS consecutive matmuls; groups g/g-1 use all 8 banks.
    GS = 4
    NG = NT // GS
    kv16d = io["kv16d"]
    with tc.tile_pool(name="kvfp", bufs=1) as kvfp, \
         tc.tile_pool(name="cvt", bufs=3) as cst, \
         tc.tile_pool(name="psC", bufs=1, space="PSUM") as psC:
        for mc in range(3):
            kvf = kvfp.tile([128, 130 * W], F16, tag=f"kvf{mc % 2}",
                            name=f"kvf{mc}")
            kvf3 = kvf[:].rearrange("p (r c) -> p r c", c=W)
            nc.vector.memset(kvf3[:, 0:1, :], 0.0)
            nc.vector.memset(kvf3[:, 129:130, :], 0.0)
            kvdm = kvd[mc]
            ps_h = {}

            def convgrp(g):
                for j in range(GS):
                    nt = g * GS + j
                    ps = psC.tile([128, PT], F32, tag=f"ps{nt % (2 * GS)}",
                                  name=f"ps_m{mc}_{nt}")
                    ps_h[nt] = ps
                    ka = cst.tile([128, PT], F16, tag="ka")
                    kb = cst.tile([64, PT], F16, tag="kb")
                    if mc == 0:
                        sa = cst.tile([128, PT], F32, tag="sa")
                        nc.sync.dma_start(sa[:], kv[0:128, sl(nt)])
                        nc.vector.tensor_copy(ka[:], sa[:])
                        sb = cst.tile([64, PT], F32, tag="sb")
                        nc.sync.dma_start(sb[:], kv[128:C, sl(nt)])
                        nc.scalar.copy(kb[:], sb[:])
                        nc.sync.dma_start(kv16d[0:128, sl(nt)], ka[:])
                        nc.sync.dma_start(kv16d[128:C, sl(nt)], kb[:])
                    else:
                        nc.sync.dma_start(ka[:], kv16d[0:128, sl(nt)])
                        nc.sync.dma_start(kb[:], kv16d[128:C, sl(nt)])
                    nc.tensor.matmul(ps[:], w1ta[:, mc * 128:(mc + 1) * 128],
                                     ka[:], start=True, stop=False,
                                     skip_group_check=True)
                    nc.tensor.matmul(ps[:], w1tb[:, mc * 128:(mc + 1) * 128],
                                     kb[:], start=False, stop=False,
                                     skip_group_check=True)
                    # conv1-only result -> kvf (pad row offset 1)
                    dst = kvf[:, 128 + nt * PT: 128 + (nt + 1) * PT]
                    nc.scalar.copy(dst, ps[:])

            def tapsgrp(g):
                ntap = len(TENSOR_TAPS)
                for ti, (dr, dc) in enumerate(TENSOR_TAPS):
                    lw = w2dgb[:, mc * ntap + ti, :]
                    for j in range(GS):
                        nt = g * GS + j
                        ps = ps_h[nt]
                        ps3 = ps[:].rearrange("p (r c) -> p r c", c=W)
                        r0 = 1 + nt * RPT + dr
                        if dc == 0:
                            dst, rhs = ps[:], kvf3[:, r0:r0 + RPT, :]
                        elif dc == 1:
                            dst = ps3[:, :, 0:W - 1]
                            rhs = kvf3[:, r0:r0 + RPT, 1:W]
                        else:
                            dst = ps3[:, :, 1:W]
                            rhs = kvf3[:, r0:r0 + RPT, 0:W - 1]
                        nc.tensor.matmul(dst, lw, rhs, start=False,
                                         stop=(ti == ntap - 1),
                                         skip_group_check=True)
                # conv1 + tensor-tap partial sums -> kvd slab (f16)
                for j in range(GS):
                    nt = g * GS + j
                    ps = ps_h.pop(nt)
                    dst = kvdm[:, sl(nt)]
                    nc.vector.tensor_copy(dst, ps[:])

            for g in range(NG):
                convgrp(g)
                if g >= 1:
                    tapsgrp(g - 1)
            tapsgrp(NG - 1)
            if debug:
                nc.sync.dma_start(io[f"dbg_conv{mc}"][:, :], kvdm[:])

            # remaining taps: in-place FMA on vector
            kvdm3 = kvdm[:].rearrange("p (r c) -> p r c", c=W)
            for (dr, dc) in FMA_TAPS:
                wi = (dr + 1) * 3 + (dc + 1)
                wsc = w2sb[:, mc * 9 + wi: mc * 9 + wi + 1]
                r0 = 1 + dr
                if dc == 0:
                    dst = kvdm3[:, :, :]
                    src = kvf3[:, r0:r0 + H, :]
                elif dc == 1:
                    dst = kvdm3[:, :, 0:W - 1]
                    src = kvf3[:, r0:r0 + H, 1:W]
                else:
                    dst = kvdm3[:, :, 1:W]
                    src = kvf3[:, r0:r0 + H, 0:W - 1]
                nc.vector.scalar_tensor_tensor(out=dst, in0=src, scalar=wsc,
                                               in1=dst, op0=OP.mult,
                                               op1=OP.add)
            if debug:
                nc.sync.dma_start(io[f"dbg_kvd{mc}"][:, :], kvdm[:])

    # ---- phase 2: k norms; scale k rows by 1/max(|k|, eps) ----
    with tc.tile_pool(name="norm", bufs=1) as npl:
        NCH = 8
        CHW = HWTOT // NCH
        kpa = npl.tile([128, NCH], F32)
        kpb = npl.tile([64, NCH], F32)
        sqs = npl.tile([128, CHW], F16)
        for i in range(NCH):
            nc.scalar.activation(sqs[:, :], kvd[0][:, sl(i, CHW)], AF.Square,
                                 accum_out=kpa[:, i:i + 1])
        for i in range(NCH):
            nc.scalar.activation(sqs[0:64, :], kvd[1][0:64, sl(i, CHW)],
                                 AF.Square, accum_out=kpb[:, i:i + 1])
        nk2a = npl.tile([128, 1], F32)
        nk2b = npl.tile([64, 1], F32)
        nc.vector.reduce_sum(nk2a[:], kpa[:], axis=mybir.AxisListType.X)
        nc.vector.reduce_sum(nk2b[:], kpb[:], axis=mybir.AxisListType.X)
        for nk2 in (nk2a, nk2b):
            nc.scalar.sqrt(nk2[:], nk2[:])
            nc.vector.tensor_scalar_max(nk2[:], nk2[:], EPS)
            nc.vector.reciprocal(nk2[:], nk2[:])
        nc.vector.tensor_scalar_mul(kvd[0][:], kvd[0][:], nk2a[:])
        nc.vector.tensor_scalar_mul(kvd[1][0:64, :], kvd[1][0:64, :], nk2b[:])

        # ---- phase 3: transposes + q load + Gram (quarter-pipelined) ----
        qpa = npl.tile([128, NT], F32)
        qpb = npl.tile([64, NT], F32)
        with tc.tile_pool(name="tp", bufs=1) as tp, \
             tc.tile_pool(name="qst", bufs=3) as qst, \
             tc.tile_pool(name="q16p", bufs=1) as q16p, \
             tc.tile_pool(name="psG", bufs=1, space="PSUM") as psG:
            G0 = psG.tile([128, C], F32, name="G0")
            G1 = psG.tile([64, C], F32, name="G1")
            for qt in range(4):
                kB = tp.tile([128, 32, C], F16, tag=f"kB{qt % 2}",
                             name=f"kB{qt}")
                nc.sync.dma_start_transpose(
                    kB[:, :, 0:128], kvd[0][:, sl(qt, QCH)])
                nc.sync.dma_start_transpose(
                    kB[:, :, 128:C], kvd[1][0:64, sl(qt, QCH)])
                q16a = q16p.tile([128, QCH], F16, tag=f"qa{qt % 2}")
                q16b = q16p.tile([64, QCH], F16, tag=f"qb{qt % 2}")
                for j in range(8):
                    nt = qt * 8 + j
                    sa = qst.tile([128, PT], F32, tag="sa")
                    nc.sync.dma_start(sa[:], q[0:128, sl(nt)])
                    nc.any.tensor_copy(q16a[:, sl(j)], sa[:])
                    nc.scalar.activation(sqs[:, 0:PT], q16a[:, sl(j)],
                                         AF.Square,
                                         accum_out=qpa[:, nt:nt + 1])
                    sb = qst.tile([64, PT], F32, tag="sb")
                    nc.sync.dma_start(sb[:], q[128:C, sl(nt)])
                    nc.any.tensor_copy(q16b[:, sl(j)], sb[:])
                    nc.scalar.activation(sqs[0:64, 0:PT], q16b[:, sl(j)],
                                         AF.Square,
                                         accum_out=qpb[:, nt:nt + 1])
                qB = tp.tile([128, 32, C], F16, tag=f"qB{qt % 2}",
                             name=f"qB{qt}")
                nc.sync.dma_start_transpose(qB[:, :, 0:128], q16a[:])
                nc.sync.dma_start_transpose(qB[:, :, 128:C], q16b[:])
                for t in range(32):
                    first = (qt == 0 and t == 0)
                    last = (qt == 3 and t == 31)
                    nc.tensor.matmul(G0[:], qB[:, t, 0:128], kB[:, t, :],
                                     start=first, stop=last,
                                     skip_group_check=True)
                    nc.tensor.matmul(G1[:], qB[:, t, 128:C], kB[:, t, :],
                                     start=first, stop=last,
                                     skip_group_check=True)

            # ---- phase 4: q-norm finalize, masked softmax, MT build ----
            nq2a = npl.tile([128, 1], F32)
            nq2b = npl.tile([64, 1], F32)
            spa = npl.tile([128, 1], F32)
            spb = npl.tile([64, 1], F32)
            nc.vector.reduce_sum(nq2a[:], qpa[:], axis=mybir.AxisListType.X)
            nc.vector.reduce_sum(nq2b[:], qpb[:], axis=mybir.AxisListType.X)
            for nq2, spx, scx in ((nq2a, spa, sca), (nq2b, spb, scb)):
                nc.scalar.sqrt(nq2[:], nq2[:])
                nc.vector.tensor_scalar_max(nq2[:], nq2[:], EPS)
                nc.vector.reciprocal(nq2[:], nq2[:])
                nc.vector.tensor_tensor(out=spx[:], in0=nq2[:], in1=scx[:],
                                        op=OP.mult)
            with tc.tile_pool(name="smax", bufs=1) as sm, \
                 tc.tile_pool(name="psT", bufs=1, space="PSUM") as psT:
                if debug:
                    gd = sm.tile([128, C], F32, tag="gdump")
                    nc.vector.tensor_copy(gd[:], G0[:])
                    nc.sync.dma_start(io["dbg_G0"][:, :], gd[:])
                    gd1 = sm.tile([64, C], F32, tag="gdump1")
                    nc.vector.tensor_copy(gd1[:], G1[:])
                    nc.sync.dma_start(io["dbg_G1"][:, :], gd1[:])
                at16 = {}
                for Gx, spx, mkx, rows, key in ((G0, spa, maska, 128, 'a'),
                                                (G1, spb, maskb, 64, 'b')):
                    lg = sm.tile([rows, C], F32, tag=f"lg{key}")
                    nc.vector.scalar_tensor_tensor(
                        out=lg[:], in0=Gx[:], scalar=spx[:], in1=mkx[:],
                        op0=OP.mult, op1=OP.add)
                    mx = sm.tile([rows, 1], F32, tag=f"mx{key}")
                    nc.vector.reduce_max(mx[:], lg[:],
                                         axis=mybir.AxisListType.X)
                    nc.vector.tensor_scalar_mul(mx[:], mx[:], -1.0)
                    ssum = sm.tile([rows, 1], F32, tag=f"ss{key}")
                    nc.scalar.activation(lg[:], lg[:], AF.Exp, bias=mx[:],
                                         accum_out=ssum[:])
                    nc.vector.reciprocal(ssum[:], ssum[:])
                    at = sm.tile([rows, C], F16, tag=f"at{key}")
                    nc.vector.tensor_scalar_mul(at[:], lg[:], ssum[:])
                    at16[key] = at
                # MT = attn^T @ Wp^T ; chunks match v slab partition ranges.
                # MTx lives on partitions 64..127 so its base partition
                # matches kvd1's v half in phase 5 (matmul requires
                # lhsT.base_partition == rhs.base_partition).
                MTxf_ps = psT.tile([128, C], F32, tag="MTx")
                MTx_ps = MTxf_ps[64:128, :]
                nc.tensor.matmul(MTx_ps, at16['a'][:, 0:64], wpta[:],
                                 start=True, stop=False)
                nc.tensor.matmul(MTx_ps, at16['b'][:, 0:64], wptb[:],
                                 start=False, stop=True)
                MTy_ps = psT.tile([128, C], F32, tag="MTy")
                nc.tensor.matmul(MTy_ps[:], at16['a'][:, 64:C], wpta[:],
                                 start=True, stop=False)
                nc.tensor.matmul(MTy_ps[:], at16['b'][:, 64:C], wptb[:],
                                 start=False, stop=True)
                MTxf = sml.tile([128, C], F16)
                MTy = sml.tile([128, C], F16)
                nc.vector.tensor_copy(MTxf[64:128, :], MTx_ps)
                nc.vector.tensor_copy(MTy[:], MTy_ps[:])
                MTx = MTxf
                if debug:
                    nc.sync.dma_start(io["dbg_at0"][:, :], at16['a'][:])
                    nc.sync.dma_start(io["dbg_at1"][:, :], at16['b'][:])
                    nc.sync.dma_start(io["dbg_MTx"][:, :], MTxf[64:128, :])
                    nc.sync.dma_start(io["dbg_MTy"][:, :], MTy[:])

    # ---- phase 5: out = MT^T @ v, streamed per pixel tile ----
    with tc.tile_pool(name="psO", bufs=2, space="PSUM") as psO, \
         tc.tile_pool(name="ost", bufs=3) as ost:
        for nt in range(NT):
            O0 = psO.tile([128, PT], F32, tag="O0")
            O1 = psO.tile([64, PT], F32, tag="O1")
            nc.tensor.matmul(O0[:], MTx[64:128, 0:128], kvd[1][64:128, sl(nt)],
                             start=True, stop=False)
            nc.tensor.matmul(O0[:], MTy[:, 0:128], kvd[2][:, sl(nt)],
                             start=False, stop=True)
            nc.tensor.matmul(O1[:], MTx[64:128, 128:C], kvd[1][64:128, sl(nt)],
                             start=True, stop=False)
            nc.tensor.matmul(O1[:], MTy[:, 128:C], kvd[2][:, sl(nt)],
                             start=False, stop=True)
            fa = ost.tile([128, PT], F32, tag="fa")
            fb = ost.tile([64, PT], F32, tag="fb")
            if nt % 2 == 0:
                nc.vector.tensor_copy(fa[:], O0[:])
                nc.scalar.copy(fb[:], O1[:])
            else:
                nc.scalar.copy(fa[:], O0[:])
                nc.vector.tensor_copy(fb[:], O1[:])
            nc.sync.dma_start(out[0:128, sl(nt)], fa[:])
            nc.sync.dma_start(out[128:C, sl(nt)], fb[:])
    _stack.close()


def build_module(debug=False):
    nc = bacc.Bacc("TRN2")
    io = {}
    io["kv"] = nc.dram_tensor("kv", [C, HWTOT], F32, kind="ExternalInput").ap()
    io["q"] = nc.dram_tensor("q", [C, HWTOT], F32, kind="ExternalInput").ap()
    io["w1t"] = nc.dram_tensor("w1t", [C, C2], F16, kind="ExternalInput").ap()
    io["w2s"] = nc.dram_tensor("w2s", [128, 27], F16, kind="ExternalInput").ap()
    io["w2dg"] = nc.dram_tensor("w2dg", [128, 3 * len(TENSOR_TAPS), 128],
                                F16, kind="ExternalInput").ap()
    io["wpt"] = nc.dram_tensor("wpt", [C, C], F16, kind="ExternalInput").ap()
    io["mask"] = nc.dram_tensor("mask", [C, C], F32, kind="ExternalInput").ap()
    io["scale192"] = nc.dram_tensor("scale192", [C, 1], F32,
                                    kind="ExternalInput").ap()
    io["out"] = nc.dram_tensor("out", [C, HWTOT], F32,
                               kind="ExternalOutput").ap()
    io["kv16d"] = nc.dram_tensor("kv16d", [C, HWTOT], F16).ap()
    if debug:
        for i in range(3):
            io[f"dbg_conv{i}"] = nc.dram_tensor(
                f"dbg_conv{i}", [128, HWTOT], F16, kind="ExternalOutput").ap()
            io[f"dbg_kvd{i}"] = nc.dram_tensor(
                f"dbg_kvd{i}", [128, HWTOT], F16, kind="ExternalOutput").ap()
        io["dbg_G0"] = nc.dram_tensor("dbg_G0", [128, C], F32,
                                      kind="ExternalOutput").ap()
        io["dbg_G1"] = nc.dram_tensor("dbg_G1", [64, C], F32,
                                      kind="ExternalOutput").ap()
        io["dbg_at0"] = nc.dram_tensor("dbg_at0", [128, C], F16,
                                       kind="ExternalOutput").ap()
        io["dbg_at1"] = nc.dram_tensor("dbg_at1", [64, C], F16,
                                       kind="ExternalOutput").ap()
        io["dbg_MTx"] = nc.dram_tensor("dbg_MTx", [64, C], F16,
                                       kind="ExternalOutput").ap()
        io["dbg_MTy"] = nc.dram_tensor("dbg_MTy", [128, C], F16,
                                       kind="ExternalOutput").ap()
    with tile.TileContext(nc) as tc:
        emit_kernel(tc, io, debug=debug)
    nc.compile()
    return nc


def prep_weights(qkv1_w, qkv2_w, proj_w, scale):
    w1 = np.asarray(qkv1_w).reshape(C2, C)
    w1t = np.ascontiguousarray(w1.T).astype(BF16NP)
    w2 = np.asarray(qkv2_w).reshape(C2, 9)
    w2s = np.zeros((128, 27), BF16NP)
    for mc in range(3):
        w2s[:, mc * 9:(mc + 1) * 9] = w2[mc * 128:(mc + 1) * 128, :]
    w2dg = np.zeros((128, 3 * len(TENSOR_TAPS), 128), BF16NP)
    for mc in range(3):
        for ti, (dr, dc) in enumerate(TENSOR_TAPS):
            wi = (dr + 1) * 3 + (dc + 1)
            wv = w2[mc * 128:(mc + 1) * 128, wi].astype(np.float64)
            if dr == 0 and dc == 0:
                # the PSUM already holds the raw conv1 output (it is
                # evacuated to kvf mid-group); bias the center tap by -1
                # so conv1*(1) + conv1*(w00-1) == conv1*w00
                wv = wv - 1.0
            np.fill_diagonal(w2dg[:, mc * len(TENSOR_TAPS) + ti, :], wv)
    wpr = np.asarray(proj_w).reshape(C, C)
    wpt = np.ascontiguousarray(wpr.T).astype(BF16NP)
    mask = np.full((C, C), -1e30, np.float32)
    for h in range(HEADS):
        mask[h * CD:(h + 1) * CD, h * CD:(h + 1) * CD] = 0.0
    scale192 = np.repeat(np.asarray(scale).reshape(HEADS), CD).astype(
        np.float32).reshape(C, 1)
    return {"w1t": w1t, "w2s": w2s, "w2dg": w2dg, "wpt": wpt,
            "mask": mask, "scale192": scale192}


_CACHED = {}


def kernel(kv, q, qkv1_w, qkv2_w, proj_w, scale):
    kv = np.asarray(kv, np.float32)
    q = np.asarray(q, np.float32)
    b = kv.shape[0]
    assert b == 8 and kv.shape[1] == C
    wts = prep_weights(qkv1_w, qkv2_w, proj_w, scale)
    if "nc" not in _CACHED:
        nc = build_module()
        nc.m = get_hw_module(nc.m)
        _CACHED["nc"] = nc
    nc = _CACHED["nc"]
    in_maps = []
    for i in range(b):
        m = {"kv": np.ascontiguousarray(kv[i].reshape(C, HWTOT)),
             "q": np.ascontiguousarray(q[i].reshape(C, HWTOT))}
        m.update(wts)
        in_maps.append(m)
    res = run_bass_kernel_spmd(nc, in_maps, core_ids=list(range(8)))
    out = np.stack([res.results[i]["out"].reshape(C, H, W) for i in range(b)])
    return out.astype(np.float32)
